# revision 31
# baseline (speedup 1.0000x reference)
"""Trainium2 Bass kernel for nn_DeepBackward (dense MLP forward + loss).

Data parallel over batch (B=32768 -> 4096 rows/core x 8 cores), activations
feature-on-partition. Sync-BN via closed-form BN0/BN1 (from global moments of
x, computed redundantly per core) and AllReduce'd sum/sumsq for BN2/BN3.

v2 (faster than the f32r baseline):
- all matmuls in bf16 (f32 rhs streams at ~2 cycles/col; bf16 at 1, and
  weight loads get FWL), weights converted/folded to bf16 on the fly
- input layers normalize straight out of PSUM (closed-form BN1 shift is
  known before the matmul) -- no spill, no stats
- hidden-layer sumsq via tensor_tensor_reduce on the bf16 spill instead of
  bn_stats on psum (DVE: ~0.4us/chunk vs ~1.4us)
- ACT stays in the reciprocal_sqrt_and_small table for the whole pipeline
  (Rsqrt for BN scale instead of Ln/Exp chains); one table switch to
  natural_log(+exp) for the final softplus, hidden under the last AllReduce
- DMA issue order: x first (moments gate the closed form), big weights
  consolidated into few descriptors and issued from the otherwise-idle
  gpsimd queue; host packs small params into one tensor
- dummy warmup AllReduce at t~0 absorbs the ~11us CC-stream cold start
- final elementwise stage restructured into ~5 fused serial ops
"""
import os
import sys

import numpy as np

sys.path.insert(0, "/opt/trn_rl_repo")

import concourse.bacc as bacc  # noqa: E402
import concourse.mybir as mybir  # noqa: E402
import concourse.tile as tile  # noqa: E402
from concourse.bass_utils import run_bass_kernel_spmd  # noqa: E402

N_CORES = 8
# bisection flags (default = conservative settings that ran on HW)
_GP_DMA = bool(int(os.environ.get("K_GP_DMA", "0")))       # DMAs on gpsimd queue
_WARM_AR = bool(int(os.environ.get("K_WARM_AR", "1")))     # warmup AllReduce
_GP_NORM = bool(int(os.environ.get("K_GP_NORM", "1")))     # gpsimd norm chunks
_TTR = bool(int(os.environ.get("K_TTR", "1")))             # tensor_tensor_reduce sumsq
_STAGE = int(os.environ.get("K_STAGE", "4"))               # pipeline cut for bisection
_STT = bool(int(os.environ.get("K_STT", "1")))             # scalar_tensor_tensor fused op
B = 32768
BC = B // N_CORES  # 4096 rows per core
H = 512
MT = 4  # m-tiles (feature tiles of 128)
KT = 4  # k-tiles
NH = 2
EPS = 1e-5
DT = 1.0 / 50.0
R = 0.05
EPSILON = 0.1
K1 = 1.0 + DT + DT * R

F32 = mybir.dt.float32
BF16 = mybir.dt.bfloat16
AL = mybir.AluOpType
AF = mybir.ActivationFunctionType

# partition-0 scalar slots (mo) -> broadcast rows (bc)
(BC_EX, BC_EF, BC_VARX, BC_COV2, BC_VARF, BC_S0Y0, BC_S0Y1, BC_S0Z, BC_BOUTY,
 BC_BOUTZ, BC_K1BY) = range(11)
BC_W = 16

# tiny packed param layout (host side must match)
(TI_G0Y0, TI_G0Y1, TI_G0Z, TI_BOUTY, TI_BOUTZ) = range(5)
TI_W = 8


def _build():
    nc = bacc.Bacc("TRN2", target_bir_lowering=False, debug=False,
                   num_devices=N_CORES)

    # ---- DRAM I/O ------------------------------------------------------
    d = {}
    d["xf"] = nc.dram_tensor("xf", [B], F32, kind="ExternalInput")
    d["slab"] = nc.dram_tensor("slab", [3, BC], F32, kind="ExternalInput")
    d["tiny"] = nc.dram_tensor("tiny", [TI_W], F32, kind="ExternalInput")
    d["pk"] = nc.dram_tensor("pk", [128 * 68], F32, kind="ExternalInput")
    for p in ("y", "z"):
        nf = 2 if p == "y" else 1
        d[f"{p}_w_in"] = nc.dram_tensor(f"{p}_w_in", [nf, H], F32, kind="ExternalInput")
        d[f"{p}_w_h"] = nc.dram_tensor(f"{p}_w_h", [NH, H, H], F32, kind="ExternalInput")
        d[f"{p}_w_out"] = nc.dram_tensor(f"{p}_w_out", [H], F32, kind="ExternalInput")
        d[f"{p}_bn_g"] = nc.dram_tensor(f"{p}_bn_g", [3, H], F32, kind="ExternalInput")
        d[f"{p}_bn_b"] = nc.dram_tensor(f"{p}_bn_b", [3, H], F32, kind="ExternalInput")
    out_partial = nc.dram_tensor("out_partial", [128, 1], F32, kind="ExternalOutput")

    with tile.TileContext(nc) as tc:
        with (
            tc.tile_pool(name="w", bufs=1) as wp,
            tc.tile_pool(name="whf", bufs=1) as whf_pool,
            tc.tile_pool(name="spill", bufs=2) as sp_pool,
            tc.tile_pool(name="rhs", bufs=32) as rhs_pool,
            tc.tile_pool(name="sqscr", bufs=1) as sq_pool,
            tc.tile_pool(name="psum", bufs=2, space="PSUM") as ps,
            tc.tile_pool(name="stats", bufs=2) as st_pool,
            tc.tile_pool(name="small", bufs=2) as sm,
            tc.tile_pool(name="fin", bufs=1) as fin,
            tc.tile_pool(name="dram", bufs=1, space="DRAM") as dram,
        ):
            # ---- constants / early tiles ------------------------------
            cm1 = wp.tile([128, 1], F32, tag="cm1", name="cm1")
            nc.vector.memset(cm1[:], -1.0)
            ceps = wp.tile([128, 1], F32, tag="ceps", name="ceps")
            nc.vector.memset(ceps[:], EPS)
            ones1 = wp.tile([128, 1], F32, tag="ones1", name="ones1")
            nc.vector.memset(ones1[:], 1.0)
            wup = wp.tile([128, 256], BF16, tag="wup", name="wup")
            nc.vector.memset(wup[:], 0.25)

            # dummy sqrt: forces the sqrt_and_others ACT table at t~0 so no
            # table load lands on the startup critical path
            dumt = wp.tile([1, 2], F32, tag="dumt", name="dumt")
            nc.vector.memset(dumt[:], 1.0)
            nc.scalar.activation(dumt[:, 1:2], dumt[:, 0:1], AF.Sqrt)

            # ---- critical input DMAs first (sync queue) ---------------
            xf_t = wp.tile([128, B // 128], F32, tag="xf", name="xf")
            nc.sync.dma_start(xf_t[:], d["xf"].ap().rearrange("(p n) -> p n", p=128))
            slab_t = wp.tile([128, 3, BC // 128], F32, tag="slab", name="slab")
            nc.sync.dma_start(
                slab_t[:], d["slab"].ap().rearrange("r (p n) -> p r n", p=128))
            x_t = slab_t[:, 0, :]
            xn_t = slab_t[:, 1, :]
            dw_t = slab_t[:, 2, :]
            tiny_t = wp.tile([1, TI_W], F32, tag="tiny", name="tiny")
            nc.sync.dma_start(tiny_t[:], d["tiny"].ap().unsqueeze(0))
            # early AR-warmup payload DMA (before the param burst, so the
            # collective trigger on gpsimd never waits long)
            ar_w = sm.tile([128, 2], F32, tag="ar_w", name="ar_w")
            nc.vector.memset(ar_w[:], 1.0)
            bi_w = dram.tile([128, 2], F32, tag="arw_i", name="arw_i")
            bo_w = dram.tile([128, 2], F32, tag="arw_o", name="arw_o",
                             addr_space="Shared")
            nc.sync.dma_start(bi_w[:], ar_w[:])
            # host-packed transposed small params: one coarse DMA instead of
            # many 4-byte-element gathers (descriptor generation took ~3us each)
            pk_t = wp.tile([128, 68], F32, tag="pk", name="pk")
            nc.sync.dma_start(pk_t[:], d["pk"].ap().rearrange("(p x) -> p x", p=128))
            w_in_f = {}
            winT = {}
            for p, nf in (("y", 2), ("z", 1)):
                w_in_f[p] = wp.tile([nf, H], F32, tag=f"winf_{p}", name=f"winf_{p}")
                nc.sync.dma_start(w_in_f[p][:], d[f"{p}_w_in"].ap())
            winT["y"] = pk_t[:, 56:64].rearrange("p (mt f) -> p mt f", f=2)
            winT["z"] = pk_t[:, 64:68].rearrange("p (mt f) -> p mt f", f=1)

            # ---- big/param DMAs on the gpsimd queue -------------------
            w_hf = {}   # (p, layer) -> [128, KT, H] f32, bufs=1 per net (reloaded)
            w_out_f = {}
            g_sb = {}
            b_sb = {}
            for p in ("y", "z"):
                # layer 0 now; layer 1 is DMA'd into the same buffer after the
                # layer-0 fold (emitting it here would block the gpsimd queue
                # on the fold, deadlocking against partition_broadcast below)
                w_hf[p] = {}
                t = whf_pool.tile([128, KT, H], F32, tag=f"whf_{p}",
                                  name=f"whf_{p}_0")
                _dmae0 = nc.gpsimd if _GP_DMA else nc.sync
                _dmae0.dma_start(
                    t[:], d[f"{p}_w_h"].ap()[0].rearrange(
                        "(kt p) m -> p kt m", p=128))
                w_hf[p][0] = t
                base = 0 if p == "y" else 24
                g_sb[p] = pk_t[:, base:base + 12].rearrange("p (l m) -> p l m", m=MT)
                b_sb[p] = pk_t[:, base + 12:base + 24].rearrange("p (l m) -> p l m", m=MT)
                w_out_f[p] = pk_t[:, (48 if p == "y" else 52):(52 if p == "y" else 56)]
            bg_sb = {}
            for p in ("y", "z"):
                bg_sb[p] = wp.tile([128, 3, MT], F32, tag=f"bg_{p}", name=f"bg_{p}")
                nc.vector.reciprocal(bg_sb[p][:], g_sb[p])
                nc.vector.tensor_tensor(out=bg_sb[p][:], in0=bg_sb[p][:],
                                        in1=b_sb[p], op=AL.mult)



            def ts(out, in0, s1, op0, s2=None, op1=None):
                if op1 is not None:
                    kw = dict(scalar2=s2, op1=op1)
                else:
                    kw = dict(scalar2=None)
                return nc.vector.tensor_scalar(out=out, in0=in0, scalar1=s1,
                                               op0=op0, **kw)

            def tt(out, a, b2, op):
                return nc.vector.tensor_tensor(out=out, in0=a, in1=b2, op=op)

            def stt(out, in0, scalar, in1, op0, op1, engine=None, accum_out=None):
                eng = engine or nc.vector
                if _STT:
                    return eng.scalar_tensor_tensor(out=out, in0=in0, scalar=scalar,
                                                    in1=in1, op0=op0, op1=op1,
                                                    accum_out=accum_out)
                shp = [int(s) for s in in0.shape]
                scr = sm.tile(shp, F32, tag="sttscr", name="sttscr", bufs=4)
                nc.vector.tensor_scalar(out=scr[:], in0=in0, scalar1=scalar,
                                        scalar2=None, op0=op0)
                return nc.vector.tensor_tensor(out=out, in0=scr[:], in1=in1, op=op1)

            # ---- global moments of x (full batch, every core) ---------
            Ff_t = wp.tile([128, B // 128], F32, tag="Ff", name="Ff")
            nc.scalar.activation(Ff_t[:], xf_t[:], AF.Relu, bias=cm1[:])
            scr_m = wp.tile([128, B // 128], F32, tag="scr_m", name="scr_m")
            acc = wp.tile([128, 8], F32, tag="acc", name="acc")
            nc.vector.reduce_sum(acc[:, 0:1], xf_t[:], axis=mybir.AxisListType.X)
            nc.vector.reduce_sum(acc[:, 1:2], Ff_t[:], axis=mybir.AxisListType.X)
            for _i, (_a, _b) in enumerate(
                    [(xf_t, xf_t), (xf_t, Ff_t), (Ff_t, Ff_t)]):
                if _TTR:
                    nc.vector.scalar_tensor_tensor(
                        out=scr_m[:], in0=_a[:], scalar=0.0, in1=_b[:],
                        op0=AL.bypass, op1=AL.mult,
                        accum_out=acc[:, 2 + _i:3 + _i])
                else:
                    nc.vector.tensor_tensor(out=scr_m[:], in0=_a[:], in1=_b[:],
                                            op=AL.mult)
                    nc.vector.reduce_sum(acc[:, 2 + _i:3 + _i], scr_m[:],
                                         axis=mybir.AxisListType.X)

            # PE warmup (bf16) first: PE busy from t~1us, trains the HAM
            # clock gate; the ones-matmul (gated on the moments) queues after
            warm_ps = ps.tile([128, 2048], F32, tag="mm", name="mm_warm")
            for wi in range(10):
                nc.tensor.matmul(warm_ps[:, (wi % 4) * 512:(wi % 4) * 512 + 256],
                                 wup[:, 0:128], wup[:, 0:256], start=True, stop=True)

            # cross-partition reduction of the 5 moment accumulators
            ps_m = ps.tile([1, 2048], F32, tag="mm", name="mm_mom")
            nc.tensor.matmul(ps_m[0:1, 0:5], ones1[:], acc[:, 0:5],
                             start=True, stop=True)
            for wi in range(14):
                nc.tensor.matmul(warm_ps[:, (wi % 4) * 512:(wi % 4) * 512 + 256],
                                 wup[:, 0:128], wup[:, 0:256], start=True, stop=True)

            t5 = wp.tile([1, 8], F32, tag="t5", name="t5")
            nc.scalar.copy(t5[:, 0:5], ps_m[0:1, 0:5])

            # partition-0 closed-form scalar chain
            invB = 1.0 / float(B)
            mo = wp.tile([1, BC_W], F32, tag="mo", name="mo")
            tE = wp.tile([1, 8], F32, tag="tE", name="tE")
            ts(tE[:, 0:5], t5[:, 0:5], invB, AL.mult)  # Ex EF Exx ExF EFF
            nc.scalar.copy(mo[:, BC_EX:BC_EF + 1], tE[:, 0:2])
            tA = wp.tile([1, 8], F32, tag="tA", name="tA")
            tt(tA[:, 0:1], tE[:, 0:1], tE[:, 0:1], AL.mult)     # Ex^2
            tt(tA[:, 1:2], tE[:, 0:1], tE[:, 1:2], AL.mult)     # Ex*EF
            tt(tA[:, 2:3], tE[:, 1:2], tE[:, 1:2], AL.mult)     # EF^2
            tt(mo[:, BC_VARX:BC_VARX + 1], tE[:, 2:3], tA[:, 0:1], AL.subtract)
            tt(tA[:, 3:4], tE[:, 3:4], tA[:, 1:2], AL.subtract)  # covxF
            ts(mo[:, BC_COV2:BC_COV2 + 1], tA[:, 3:4], 2.0, AL.mult)
            # varF into the slot right after varx is NOT adjacent; compute to
            # tA4 first, then place [varx, varF] adjacently for one Rsqrt
            tt(tA[:, 4:5], tE[:, 4:5], tA[:, 2:3], AL.subtract)  # varF
            nc.scalar.copy(mo[:, BC_VARF:BC_VARF + 1], tA[:, 4:5])
            vv = wp.tile([1, 6], F32, tag="vv", name="vv")
            nc.scalar.copy(vv[:, 0:1], mo[:, BC_VARX:BC_VARX + 1])
            nc.scalar.copy(vv[:, 1:2], mo[:, BC_VARF:BC_VARF + 1])
            nc.scalar.activation(vv[:, 4:6], vv[:, 0:2], AF.Sqrt,
                                 bias=ceps[0:1, :])
            nc.vector.reciprocal(vv[:, 2:4], vv[:, 4:6])  # [rsx, rsF]
            tt(mo[:, BC_S0Y0:BC_S0Y0 + 1], vv[:, 2:3], tiny_t[:, TI_G0Y0:TI_G0Y0 + 1], AL.mult)
            tt(mo[:, BC_S0Y1:BC_S0Y1 + 1], vv[:, 3:4], tiny_t[:, TI_G0Y1:TI_G0Y1 + 1], AL.mult)
            tt(mo[:, BC_S0Z:BC_S0Z + 1], vv[:, 2:3], tiny_t[:, TI_G0Z:TI_G0Z + 1], AL.mult)
            nc.scalar.copy(mo[:, BC_BOUTY:BC_BOUTY + 2], tiny_t[:, TI_BOUTY:TI_BOUTY + 2])
            ts(mo[:, BC_K1BY:BC_K1BY + 1], tiny_t[:, TI_BOUTY:TI_BOUTY + 1], K1, AL.mult)

            # partition broadcast via a K=1 ones-matmul on the (warm, idle)
            # PE: gpsimd.partition_broadcast triggers an ~11us Q7 LOAD_LIB
            # microcode swap that stalled the whole startup chain
            ones_row = wp.tile([1, 128], F32, tag="ones_row", name="ones_row")
            nc.vector.memset(ones_row[:], 1.0)
            ps_bc = ps.tile([128, 2048], F32, tag="mm", name="mm_bc")
            nc.tensor.matmul(ps_bc[:, 0:BC_W], ones_row[:], mo[:],
                             start=True, stop=True)
            bc = wp.tile([128, BC_W], F32, tag="bc", name="bc")
            nc.scalar.copy(bc[:], ps_bc[:, 0:BC_W])
            # warmup AllReduce (absorbs CC-stream cold start); after the
            # broadcast so it never delays the closed-form chain
            nc.gpsimd.collective_compute(
                "AllReduce", AL.add, replica_groups=[list(range(N_CORES))],
                ins=[bi_w.opt()], outs=[bo_w.opt()])

            # ---- bf16 input weights -----------------------------------
            # y: h0 rows are pre-scaled by s0 -> plain convert.
            # z: rides h0's row0 (s0y0*x), so fold s0z/s0y0 into the weights
            # and pad with a zero row to share base partition 0.
            w_in_bf = {}
            w_in_bf["y"] = wp.tile([2, H], BF16, tag="winbf_y", name="winbf_y")
            nc.scalar.copy(w_in_bf["y"][:], w_in_f["y"][:])
            rec = wp.tile([1, 2], F32, tag="rec", name="rec")
            nc.vector.reciprocal(rec[:, 0:1], mo[:, BC_S0Y0:BC_S0Y0 + 1])
            tt(rec[:, 1:2], rec[:, 0:1], mo[:, BC_S0Z:BC_S0Z + 1], AL.mult)
            w_in_bf["z"] = wp.tile([2, H], BF16, tag="winbf_z", name="winbf_z")
            # engines cannot address partition base 1: zero both rows first,
            # then overwrite row 0 (row 1 stays zero)
            nc.vector.memset(w_in_bf["z"][:], 0.0)
            nc.vector.tensor_scalar(out=w_in_bf["z"][0:1, :], in0=w_in_f["z"][:],
                                    scalar1=rec[:, 1:2], scalar2=None, op0=AL.mult)

            # ---- h0 rows (bf16, via DRAM bounce) ----------------------
            Fx = fin.tile([128, BC // 128], F32, tag="Fx", name="Fx")
            nc.scalar.activation(Fx[:], x_t, AF.Relu, bias=cm1[:])
            rows3 = fin.tile([128, 2, BC // 128], BF16, tag="rows3", name="rows3")
            nc.vector.tensor_scalar(out=rows3[:, 0, :], in0=x_t,
                                    scalar1=bc[:, BC_S0Y0:BC_S0Y0 + 1],
                                    scalar2=None, op0=AL.mult)
            nc.vector.tensor_scalar(out=rows3[:, 1, :], in0=Fx[:],
                                    scalar1=bc[:, BC_S0Y1:BC_S0Y1 + 1],
                                    scalar2=None, op0=AL.mult)
            h0_dram = dram.tile([2, BC], BF16, tag="h0d", name="h0d")
            nc.sync.dma_start(h0_dram.rearrange("r (p n) -> p r n", p=128), rows3[:])
            h0 = wp.tile([2, BC], BF16, tag="h0", name="h0")
            nc.sync.dma_start(h0[:], h0_dram)

            # ---- closed-form BN1 scale/shift per net ------------------
            cvec = {}

            def closed_form_bn1(p):
                w0 = sm.tile([128, MT], F32, tag=f"cf_w0_{p}", name=f"cf_w0_{p}")
                mu = sm.tile([128, MT], F32, tag=f"cf_mu_{p}", name=f"cf_mu_{p}")
                var = sm.tile([128, MT], F32, tag=f"cf_var_{p}", name=f"cf_var_{p}")
                tmp = sm.tile([128, MT], F32, tag=f"cf_tmp_{p}", name=f"cf_tmp_{p}")
                if p == "y":
                    w1 = sm.tile([128, MT], F32, tag="cf_w1_y", name="cf_w1_y")
                    nc.vector.tensor_scalar(out=w0[:], in0=winT["y"][:, :, 0],
                                            scalar1=bc[:, BC_S0Y0:BC_S0Y0 + 1],
                                            scalar2=None, op0=AL.mult)
                    nc.vector.tensor_scalar(out=w1[:], in0=winT["y"][:, :, 1],
                                            scalar1=bc[:, BC_S0Y1:BC_S0Y1 + 1],
                                            scalar2=None, op0=AL.mult)
                    nc.vector.tensor_scalar(out=mu[:], in0=w0[:],
                                            scalar1=bc[:, BC_EX:BC_EX + 1],
                                            scalar2=None, op0=AL.mult)
                    nc.vector.tensor_scalar(out=tmp[:], in0=w1[:],
                                            scalar1=bc[:, BC_EF:BC_EF + 1],
                                            scalar2=None, op0=AL.mult)
                    tt(mu[:], mu[:], tmp[:], AL.add)
                    tt(var[:], w0[:], w0[:], AL.mult)
                    nc.vector.tensor_scalar(out=var[:], in0=var[:],
                                            scalar1=bc[:, BC_VARX:BC_VARX + 1],
                                            scalar2=None, op0=AL.mult)
                    tt(tmp[:], w0[:], w1[:], AL.mult)
                    nc.vector.tensor_scalar(out=tmp[:], in0=tmp[:],
                                            scalar1=bc[:, BC_COV2:BC_COV2 + 1],
                                            scalar2=None, op0=AL.mult)
                    tt(var[:], var[:], tmp[:], AL.add)
                    tt(tmp[:], w1[:], w1[:], AL.mult)
                    nc.vector.tensor_scalar(out=tmp[:], in0=tmp[:],
                                            scalar1=bc[:, BC_VARF:BC_VARF + 1],
                                            scalar2=None, op0=AL.mult)
                    tt(var[:], var[:], tmp[:], AL.add)
                else:
                    nc.vector.tensor_scalar(out=w0[:], in0=winT["z"][:, :, 0],
                                            scalar1=bc[:, BC_S0Z:BC_S0Z + 1],
                                            scalar2=None, op0=AL.mult)
                    nc.vector.tensor_scalar(out=mu[:], in0=w0[:],
                                            scalar1=bc[:, BC_EX:BC_EX + 1],
                                            scalar2=None, op0=AL.mult)
                    tt(var[:], w0[:], w0[:], AL.mult)
                    nc.vector.tensor_scalar(out=var[:], in0=var[:],
                                            scalar1=bc[:, BC_VARX:BC_VARX + 1],
                                            scalar2=None, op0=AL.mult)
                rs = sm.tile([128, MT], F32, tag=f"cf_rs_{p}", name=f"cf_rs_{p}")
                sq = sm.tile([128, MT], F32, tag=f"cf_sq_{p}", name=f"cf_sq_{p}")
                s_t = sm.tile([128, MT], F32, tag=f"cf_s_{p}", name=f"cf_s_{p}")
                c_t = st_pool.tile([128, MT], F32, tag=f"c1_{p}", name=f"c1_{p}")
                nc.scalar.activation(sq[:], var[:], AF.Sqrt, bias=ceps[:])
                nc.vector.reciprocal(rs[:], sq[:])
                tt(s_t[:], rs[:], g_sb[p][:, 0, :], AL.mult)
                tt(tmp[:], bg_sb[p][:, 0, :], sq[:], AL.mult)
                tt(c_t[:], tmp[:], mu[:], AL.subtract)
                sc_t = st_pool.tile([128, MT], F32, tag=f"sc1_{p}", name=f"sc1_{p}")
                tt(sc_t[:], s_t[:], c_t[:], AL.mult)
                return c_t, s_t, sc_t

            # bf16 folded hidden weights
            w_bf = {p: wp.tile([128, NH, KT, H], BF16, tag=f"wbf_{p}",
                               name=f"wbf_{p}") for p in ("y", "z")}

            def convert_w(p, layer):
                nc.scalar.activation(
                    w_bf[p][:, layer, :, :].rearrange("p a b -> p (a b)"),
                    w_hf[p][layer][:].rearrange("p a b -> p (a b)"), AF.Copy)

            c1y, s1y, sc1y = closed_form_bn1("y")
            c1z, s1z, sc1z = closed_form_bn1("z")
            cvec[("y", 1)] = (sc1y, s1y)
            cvec[("z", 1)] = (sc1z, s1z)
            # plain bf16 weight converts (BN scale now lives in the norm op)
            w_out_bf = {}
            for p in ("y", "z"):
                w_out_bf[p] = wp.tile([128, KT], BF16, tag=f"woutbf_{p}",
                                      name=f"woutbf_{p}")
                nc.vector.tensor_scalar(out=w_out_bf[p][:], in0=w_out_f[p],
                                        scalar1=1.0, scalar2=None, op0=AL.mult)


            # ---- norm helper: ACT only; applies the BN scale too:
            # s*relu(v+c) = relu(s*v + s*c)  (s > 0)
            def norm_chunk(out_ap, in_ap, sc_ap, s_ap, allow_gpsimd=True):
                nc.scalar.activation(out_ap, in_ap, AF.Relu, bias=sc_ap,
                                     scale=s_ap)

            # ---- input layer: matmul + norm straight from PSUM --------
            def input_layer(p):
                sc_t, s_t = cvec[(p, 1)]
                rhs_tiles = {}
                for half in range(2):
                    for mt in range(MT):
                        pt = ps.tile([128, 2048], F32, tag="mm", name="mm")
                        for n in range(4):
                            c0 = half * 2048 + n * 512
                            nc.tensor.matmul(
                                pt[:, n * 512:(n + 1) * 512],
                                w_in_bf[p][:, mt * 128:(mt + 1) * 128],
                                h0[:, c0:c0 + 512],
                                start=True, stop=True)
                        for sub in range(2):
                            q = half * 2 + sub
                            rt = rhs_pool.tile([128, 1024], BF16, tag="rhs", name="rhs")
                            norm_chunk(rt[:], pt[:, sub * 1024:(sub + 1) * 1024],
                                       sc_t[:, mt:mt + 1], s_t[:, mt:mt + 1],
                                       allow_gpsimd=False)
                            rhs_tiles[(mt, q)] = rt
                return rhs_tiles

            # ---- hidden layer, split into mm and post phases ----------
            # (post work depends on the AllReduce; emitting it as a separate
            # phase lets the other net's spills precede it on the in-order
            # ACT queue, so the AR wait never blocks those drains)
            def hidden_mm(p, layer, rhs_tiles, bn_idx):
                spill = sp_pool.tile([128, MT, BC], BF16, tag="spill", name="spill")
                acc_s = st_pool.tile([128, 8], F32, tag=f"acc_{p}", name=f"acc_{p}")
                sq_s = st_pool.tile([128, 8], F32, tag=f"sq_{p}", name=f"sq_{p}")
                for half in range(2):
                    for mt in range(MT):
                        pt = ps.tile([128, 2048], F32, tag="mm", name="mm")
                        for kt in range(KT):
                            for n in range(4):
                                q = half * 2 + n // 2
                                rt = rhs_tiles[(kt, q)]
                                nc.tensor.matmul(
                                    pt[:, n * 512:(n + 1) * 512],
                                    w_bf[p][:, layer, kt, mt * 128:(mt + 1) * 128],
                                    rt[:, (n % 2) * 512:(n % 2 + 1) * 512],
                                    start=(kt == 0), stop=(kt == KT - 1))
                        i = mt * 2 + half
                        sl = spill[:, mt, half * 2048:(half + 1) * 2048]
                        nc.scalar.activation(sl, pt[:], AF.Copy,
                                             accum_out=acc_s[:, i:i + 1])
                        scr = sq_pool.tile([128, 2048], F32, tag="sqs", name="sqs")
                        nc.vector.scalar_tensor_tensor(
                            out=scr[:], in0=sl, scalar=0.0, in1=sl,
                            op0=AL.bypass, op1=AL.mult,
                            accum_out=sq_s[:, i:i + 1])
                # combine chunk stats -> [128, MT, 2] -> AllReduce
                ar_in = sm.tile([128, MT, 2], F32, tag="ar_in", name="ar_in")
                accv = acc_s[:].rearrange("p (mt h) -> p mt h", h=2)
                sqv = sq_s[:].rearrange("p (mt h) -> p mt h", h=2)
                tt(ar_in[:, :, 0], accv[:, :, 0], accv[:, :, 1], AL.add)
                tt(ar_in[:, :, 1], sqv[:, :, 0], sqv[:, :, 1], AL.add)
                bi = dram.tile([128, MT, 2], F32, tag=f"ari_{p}{bn_idx}",
                               name=f"ari_{p}{bn_idx}")
                bo = dram.tile([128, MT, 2], F32, tag=f"aro_{p}{bn_idx}",
                               name=f"aro_{p}{bn_idx}", addr_space="Shared")
                nc.sync.dma_start(bi[:], ar_in[:])
                nc.gpsimd.collective_compute(
                    "AllReduce", AL.add, replica_groups=[list(range(N_CORES))],
                    ins=[bi.opt()], outs=[bo.opt()])
                return spill, bo

            def hidden_post(p, layer, spill, bo, bn_idx, last):
                sums_g = sm.tile([128, MT, 2], F32, tag="sums_g", name="sums_g")
                nc.sync.dma_start(sums_g[:], bo[:])
                muex = sm.tile([128, MT, 2], F32, tag="muex", name="muex")
                var = sm.tile([128, MT], F32, tag="var", name="var")
                tmp = sm.tile([128, MT], F32, tag="tmp", name="tmp")
                rs = sm.tile([128, MT], F32, tag="rs_h", name="rs_h")
                sq = sm.tile([128, MT], F32, tag="sq_h", name="sq_h")
                s_t = sm.tile([128, MT], F32, tag="s_t", name="s_t")
                c_t = st_pool.tile([128, MT], F32, tag=f"c_{p}", name=f"c_{p}")
                ts(muex[:], sums_g[:], 1.0 / float(B), AL.mult)
                mu = muex[:, :, 0]
                tt(tmp[:], mu, mu, AL.mult)
                tt(var[:], muex[:, :, 1], tmp[:], AL.subtract)
                nc.scalar.activation(sq[:], var[:], AF.Sqrt, bias=ceps[:])
                nc.vector.reciprocal(rs[:], sq[:])
                tt(s_t[:], rs[:], g_sb[p][:, bn_idx, :], AL.mult)
                tt(tmp[:], bg_sb[p][:, bn_idx, :], sq[:], AL.mult)
                tt(c_t[:], tmp[:], mu, AL.subtract)
                sc_t = st_pool.tile([128, MT], F32, tag=f"sc_{p}", name=f"sc_{p}")
                tt(sc_t[:], s_t[:], c_t[:], AL.mult)
                # norm the spill into next rhs tiles (q-major: first matmuls
                # of the next stage unblock after 4 chunks)
                rhs_next = {}
                for q in range(4):
                    for kt in range(KT):
                        rt = rhs_pool.tile([128, 1024], BF16, tag="rhs", name="rhs")
                        norm_chunk(rt[:], spill[:, kt, q * 1024:(q + 1) * 1024],
                                   sc_t[:, kt:kt + 1], s_t[:, kt:kt + 1])
                        rhs_next[(kt, q)] = rt
                return rhs_next

            # ---- out layer: [1, BC] row via DRAM bounce ---------------
            def out_layer(p, rhs_tiles, dst128):
                row = dram.tile([BC], F32, tag=f"row_{p}", name=f"row_{p}")
                for half in range(2):
                    pt = ps.tile([128, 2048], F32, tag="mm", name="mm")
                    for kt in range(KT):
                        for n in range(4):
                            q = half * 2 + n // 2
                            rt = rhs_tiles[(kt, q)]
                            nc.tensor.matmul(
                                pt[0:1, n * 512:(n + 1) * 512],
                                w_out_bf[p][:, kt:kt + 1],
                                rt[:, (n % 2) * 512:(n % 2 + 1) * 512],
                                start=(kt == 0), stop=(kt == KT - 1))
                    orow = sm.tile([1, 2048], F32, tag="orow", name="orow", bufs=2)
                    # split the single-partition drain across ACT and DVE
                    nc.scalar.copy(orow[:, 0:1024], pt[0:1, 0:1024])
                    ts(orow[:, 1024:2048], pt[0:1, 1024:2048], 0.0, AL.add)
                    nc.sync.dma_start(
                        row[half * 2048:(half + 1) * 2048].unsqueeze(0), orow[:])
                # bounce back as [128, 32]
                nc.sync.dma_start(dst128[:], row.rearrange("(p n) -> p n", p=128))

            # ---- emit the pipeline (phase-interleaved) ----------------
            if _STAGE >= 0:
                rhs_y = input_layer("y")
                rhs_z = input_layer("z")
            # plain bf16 converts (no BN-scale dependency -> run early)
            convert_w("y", 0)
            convert_w("z", 0)
            for p in ("y", "z"):
                t = whf_pool.tile([128, KT, H], F32, tag=f"whf_{p}",
                                  name=f"whf_{p}_1")
                nc.sync.dma_start(
                    t[:], d[f"{p}_w_h"].ap()[1].rearrange(
                        "(kt p) m -> p kt m", p=128))
                w_hf[p][1] = t
            convert_w("y", 1)
            convert_w("z", 1)
            if _STAGE >= 1:
                sp_y1, bo_y1 = hidden_mm("y", 0, rhs_y, 1)
                sp_z1, bo_z1 = hidden_mm("z", 0, rhs_z, 1)
                rhs_y = hidden_post("y", 0, sp_y1, bo_y1, 1, last=False)
            if _STAGE >= 2:
                sp_y2, bo_y2 = hidden_mm("y", 1, rhs_y, 2)
            if _STAGE >= 1:
                rhs_z = hidden_post("z", 0, sp_z1, bo_z1, 1, last=False)
            if _STAGE >= 2:
                sp_z2, bo_z2 = hidden_mm("z", 1, rhs_z, 2)
                rhs_y = hidden_post("y", 1, sp_y2, bo_y2, 2, last=True)

            # final-stage prep that does not depend on y/z rows; emitted here
            # so the ACT queue work (Fn) precedes the table switch
            # y_t = (mlp + b_out) + F_x, so the Fx terms collapse to
            # -(1+DT*R)*Fx and u = F - y_t = -(y128 + bouty)
            Fn = fin.tile([128, BC // 128], F32, tag="Fn", name="Fn")
            nc.scalar.activation(Fn[:], xn_t, AF.Relu, bias=cm1[:])
            FnDtF = fin.tile([128, BC // 128], F32, tag="FnDtF", name="FnDtF")
            stt(FnDtF[:], Fx[:], -(1.0 + DT * R), Fn[:], AL.mult, AL.add)
            FnDtF2 = fin.tile([128, BC // 128], F32, tag="FnDtF2", name="FnDtF2")
            nc.vector.tensor_scalar(out=FnDtF2[:], in0=FnDtF[:],
                                    scalar1=bc[:, BC_K1BY:BC_K1BY + 1],
                                    scalar2=None, op0=AL.subtract)

            y128 = fin.tile([128, BC // 128], F32, tag="y128", name="y128")
            if _STAGE >= 3:
                out_layer("y", rhs_y, y128)

            # ACT table switches (exp, then ln) land here, under the z-net's
            # last AllReduce / zOut matmul window; the ln set keeps
            # abs/square/copy for everything after
            nc.scalar.activation(dumt[:, 1:2], dumt[:, 0:1], AF.Exp)

            # y-side final chain, hidden under z's AllReduce + zOut matmuls
            negu = fin.tile([128, BC // 128], F32, tag="negu", name="negu")
            nc.vector.tensor_scalar(out=negu[:], in0=y128[:],
                                    scalar1=bc[:, BC_BOUTY:BC_BOUTY + 1],
                                    scalar2=None, op0=AL.add)
            e1 = fin.tile([128, BC // 128], F32, tag="e1", name="e1")
            nc.scalar.activation(e1[:], negu[:], AF.Exp)
            sp_t = fin.tile([128, BC // 128], F32, tag="sp_t", name="sp_t")
            nc.scalar.activation(sp_t[:], e1[:], AF.Ln, bias=1.0)
            Ay = fin.tile([128, BC // 128], F32, tag="Ay", name="Ay")
            ts(Ay[:], y128[:], -K1, AL.mult)
            tt(Ay[:], Ay[:], FnDtF2[:], AL.add)
            Cy = fin.tile([128, BC // 128], F32, tag="Cy", name="Cy")
            stt(Cy[:], sp_t[:], DT, Ay[:], AL.mult, AL.add)

            if _STAGE >= 2:
                rhs_z = hidden_post("z", 1, sp_z2, bo_z2, 2, last=True)
            z128 = fin.tile([128, BC // 128], F32, tag="z128", name="z128")
            if _STAGE >= 3:
                out_layer("z", rhs_z, z128)
            else:
                nc.vector.memset(y128[:], 0.1)
                nc.vector.memset(z128[:], 0.1)

            # z-side tail
            az = fin.tile([128, BC // 128], F32, tag="az", name="az")
            nc.scalar.activation(az[:], z128[:], AF.Abs,
                                 bias=bc[:, BC_BOUTZ:BC_BOUTZ + 1])
            zdw = fin.tile([128, BC // 128], F32, tag="zdw", name="zdw")
            stt(zdw[:], z128[:], bc[:, BC_BOUTZ:BC_BOUTZ + 1], dw_t,
                AL.add, AL.mult)
            Dz = fin.tile([128, BC // 128], F32, tag="Dz", name="Dz")
            stt(Dz[:], az[:], -EPSILON * DT, zdw[:], AL.mult, AL.subtract)
            tempd = fin.tile([128, BC // 128], F32, tag="tempd", name="tempd")
            tt(tempd[:], Cy[:], Dz[:], AL.add)
            scrf = fin.tile([128, BC // 128], F32, tag="scrf", name="scrf")
            partial = fin.tile([128, 1], F32, tag="partial", name="partial")
            nc.scalar.activation(scrf[:], tempd[:], AF.Square, accum_out=partial[:])
            nc.sync.dma_start(out_partial.ap(), partial[:])

    nc.compile()
    return nc


_NC = None


def _get_nc():
    global _NC
    if _NC is None:
        _NC = _build()
    return _NC


def kernel(**inputs):
    nc = _get_nc()
    x = np.ascontiguousarray(inputs["x"], dtype=np.float32).reshape(B)
    x_next = np.ascontiguousarray(inputs["x_next"], dtype=np.float32).reshape(B)
    dw = np.ascontiguousarray(inputs["dw"], dtype=np.float32).reshape(B)

    tiny = np.zeros(TI_W, np.float32)
    tiny[TI_G0Y0] = np.float32(inputs["y_bn0_g"].reshape(-1)[0])
    tiny[TI_G0Y1] = np.float32(inputs["y_bn0_g"].reshape(-1)[1])
    tiny[TI_G0Z] = np.float32(inputs["z_bn0_g"].reshape(-1)[0])
    tiny[TI_BOUTY] = np.float32(inputs["y_b_out"].reshape(-1)[0])
    tiny[TI_BOUTZ] = np.float32(inputs["z_b_out"].reshape(-1)[0])

    pk = np.zeros((128, 68), np.float32)
    yg = np.ascontiguousarray(inputs["y_bn_g"], np.float32).reshape(3, MT, 128)
    yb = np.ascontiguousarray(inputs["y_bn_b"], np.float32).reshape(3, MT, 128)
    zg = np.ascontiguousarray(inputs["z_bn_g"], np.float32).reshape(3, MT, 128)
    zb = np.ascontiguousarray(inputs["z_bn_b"], np.float32).reshape(3, MT, 128)
    pk[:, 0:12] = yg.transpose(2, 0, 1).reshape(128, 12)
    pk[:, 12:24] = yb.transpose(2, 0, 1).reshape(128, 12)
    pk[:, 24:36] = zg.transpose(2, 0, 1).reshape(128, 12)
    pk[:, 36:48] = zb.transpose(2, 0, 1).reshape(128, 12)
    pk[:, 48:52] = np.ascontiguousarray(inputs["y_W_out"], np.float32).reshape(KT, 128).T
    pk[:, 52:56] = np.ascontiguousarray(inputs["z_W_out"], np.float32).reshape(KT, 128).T
    pk[:, 56:64] = np.ascontiguousarray(inputs["y_W_in"], np.float32).reshape(
        2, MT, 128).transpose(2, 1, 0).reshape(128, 8)
    pk[:, 64:68] = np.ascontiguousarray(inputs["z_W_in"], np.float32).reshape(
        1, MT, 128).transpose(2, 1, 0).reshape(128, 4)

    common = {
        "xf": x,
        "tiny": tiny,
        "pk": pk.reshape(-1),
        "y_w_in": np.ascontiguousarray(inputs["y_W_in"], np.float32),
        "y_w_h": np.ascontiguousarray(inputs["y_Wh"], np.float32),
        "y_w_out": np.ascontiguousarray(inputs["y_W_out"], np.float32).reshape(H),
        "y_bn_g": np.ascontiguousarray(inputs["y_bn_g"], np.float32),
        "y_bn_b": np.ascontiguousarray(inputs["y_bn_b"], np.float32),
        "z_w_in": np.ascontiguousarray(inputs["z_W_in"], np.float32),
        "z_w_h": np.ascontiguousarray(inputs["z_Wh"], np.float32),
        "z_w_out": np.ascontiguousarray(inputs["z_W_out"], np.float32).reshape(H),
        "z_bn_g": np.ascontiguousarray(inputs["z_bn_g"], np.float32),
        "z_bn_b": np.ascontiguousarray(inputs["z_bn_b"], np.float32),
    }
    in_maps = []
    for c in range(N_CORES):
        sl = slice(c * BC, (c + 1) * BC)
        m = dict(common)
        m["slab"] = np.ascontiguousarray(
            np.stack([x[sl], x_next[sl], dw[sl]]), np.float32)
        in_maps.append(m)

    res = run_bass_kernel_spmd(nc, in_maps, core_ids=list(range(N_CORES)))
    total = np.float64(0.0)
    for c in range(N_CORES):
        total += res.results[c]["out_partial"].astype(np.float64).sum()
    return np.float32(total / B)
